# revision 23
# baseline (speedup 1.0000x reference)
"""Trainium2 Bass kernel for the sparse segment-softmax attention module.

Math: the reference computes, per nnz k,
    out[k] = segment_softmax((q1[b,i] + q2[b,j]) . v)  over segments (b, i).
Within a segment (fixed b, i), the q1[b,i].v term is constant and cancels in
softmax (shift invariance), as does the b2.v bias constant.  Hence
    out[k] = exp(u2[b, j_k]) / sum_{d in seg} exp(u2[b, j_d]),
    u2[b, n] = t2[b, n, :] . g,   g = W2^T v.

The baseline gathered exp(u2)[j] with the pool-engine IndirectCopy, which
is bound at ~28 ns per index per 16-partition group (~115 us for the 32768
gathers each NeuronCore owns).  ap_gather and dma_gather hit the same or
worse serial rates (measured).  This kernel instead does the gather on the
idle PE: the indices are kernel inputs, so the host uploads, per 128-nnz
block, a bf16 one-hot STATIONARY [128c x 128p] whose column p selects row
j%128; one LoadStationary+matmul against Ecol [128c x 4] (Ecol[c, hi] =
exp(u2[128*hi + c])) yields psum[p, 4*t+hi] = exp(u2[128*hi + L[p+128t]])
— 128 gathered candidate quads in ~130 PE cycles (~0.43 ns/value).  A DVE
multiply with a host-built hi-mask and a width-4 reduce selects the right
quadrant.  Per NeuronCore (2 batches): 256 stationary matmuls ~ 14 us on
the PE, overlapped with the one-hot upload stream.

Pipeline per core (2 of the 16 batches):
  - stream t2 shard (bf16); DVE mult by g-broadcast + ACT/DVE row-reduce
    -> u2acc [128, 4] (u2acc[p, t] = u2[128t + p]); ACT exp -> Ecol bf16.
  - 128 one-hot stationary matmuls per batch -> psum [128, 512].
  - DVE: psum * himask, reduce width-4 -> C [128, 128]; windowed segment
    softmax (sum 32, reciprocal, multiply); store.  Host applies the fixed
    inverse layout permutation.
"""

import os
from contextlib import ExitStack

import numpy as np

B = 16
N1 = 512
N2 = 512
F2 = 1024
DEG = 32
NNZ = B * N1 * DEG
NCORES = 8
BPC = B // NCORES  # batches per core
NBLK = 128  # one-hot blocks per batch (128 nnz each)
CH = 64  # one-hot contract height (j % CH selects the row)
NHI = 512 // CH  # quadrant count for the mask select

_CACHE: dict = {}


def _build_program():
    import concourse.bacc as bacc
    import concourse.mybir as mybir
    import concourse.tile as tile

    fp32 = mybir.dt.float32
    bf16 = mybir.dt.bfloat16
    fp8 = mybir.dt.float8e4

    nc = bacc.Bacc("TRN2", target_bir_lowering=False, debug=False)

    t2t = nc.dram_tensor("t2t", [BPC, F2, N2], fp8, kind="ExternalInput")
    gcol = nc.dram_tensor("gcol", [128, 8], fp8, kind="ExternalInput")
    # oh: one-hot stationaries, c-major: oh[b, c, 128t+p] = (J[p+128t] % CH == c)
    oh = nc.dram_tensor("oh", [BPC, CH, NBLK * 128], fp8, kind="ExternalInput")
    # hm: hi-quadrant mask, hm[p, NHI*t+hi] = (J[p+128t] // CH == hi)
    hm = nc.dram_tensor("hm", [BPC, 128, NHI * NBLK], fp8, kind="ExternalInput")
    out = nc.dram_tensor("out", [BPC, 128, 128], fp32, kind="ExternalOutput")

    with tile.TileContext(nc) as tc, ExitStack() as ctx:
        constp = ctx.enter_context(tc.tile_pool(name="const", bufs=1))
        t2p = ctx.enter_context(tc.tile_pool(name="t2p", bufs=4))
        prodp = ctx.enter_context(tc.tile_pool(name="prodp", bufs=3))
        ohp = ctx.enter_context(tc.tile_pool(name="ohp", bufs=1))
        smallp = ctx.enter_context(tc.tile_pool(name="small", bufs=2))
        psum_p = ctx.enter_context(tc.tile_pool(name="psg", bufs=1, space="PSUM"))
        psum_u = ctx.enter_context(tc.tile_pool(name="psu", bufs=2, space="PSUM"))

        # g columns (x256, fp8) at the head of the sync ring.
        g_sb = constp.tile([128, 8], fp8)
        nc.sync.dma_start(g_sb[:], gcol[:])
        hm_tiles = []
        for b in range(BPC):
            hm_t = constp.tile([128, NHI * NBLK], fp8, tag=f"hm{b}", name=f"hm{b}")
            nc.scalar.dma_start(hm_t[:], hm[b])
            hm_tiles.append(hm_t)

        # transposed-t2 stream on the scalar ring (1 MB fp8), batch 0
        # first; the sync ring is dedicated to the one-hot stream.
        t2_tiles = {}
        for b in range(BPC):
            for q in range(8):
                t2q = t2p.tile([128, N2], fp8, tag=f"t2_{b}_{q}", name=f"t2_{b}_{q}")
                nc.scalar.dma_start(t2q[:], t2t[b, 128 * q : 128 * (q + 1), :])
                t2_tiles[(b, q)] = t2q

        # One-hot stationaries: the sync ring carries ONLY this stream,
        # as four 1 MB chunk tiles per batch so each 32-block group of
        # matmuls starts as soon as its chunk lands.
        oh_tiles = {}
        OHCHUNK = NBLK * 128 // 4
        for b in range(BPC):
            eng = nc.sync if b == 0 else nc.scalar
            for h in range(4):
                oht = ohp.tile(
                    [CH, OHCHUNK], fp8, tag=f"oh{b}_{h}", name=f"oh{b}_{h}"
                )
                eng.dma_start(
                    oht[:], oh[b][:, h * OHCHUNK : (h + 1) * OHCHUNK]
                )
                oh_tiles[(b, h)] = oht

        # u2 on the PE: 256*u2[128t+p] = sum_q t2T[:, n].g -> upsum[p, t].
        ecols = []
        for b in range(BPC):
            upsum = psum_u.tile([128, 4], fp32, tag="upsum")
            for t in range(4):
                for q in range(8):
                    nc.tensor.matmul(
                        upsum[:, t : t + 1],
                        t2_tiles[(b, q)][:, 128 * t : 128 * (t + 1)],
                        g_sb[:, q : q + 1],
                        start=(q == 0),
                        stop=(q == 7),
                    )
            u2acc = smallp.tile([128, 4], fp32, tag="u2acc")
            nc.scalar.copy(u2acc[:], upsum[:])
            # re-lay u2acc [128, 4] -> [CH, NHI] h-major: u2[n] = u2acc[p, t]
            # with n = 128t + CH*h + c lands at column 4h + t
            u2r = smallp.tile([CH, NHI], fp32, tag=f"u2r{b}", name=f"u2r{b}")
            nc.gpsimd.dma_start(u2r[:, 0:4], u2acc[0:CH, :])
            nc.gpsimd.dma_start(u2r[:, 4:8], u2acc[CH:128, :])
            ecol = smallp.tile([CH, NHI], bf16, tag=f"ecol{b}", name=f"ecol{b}")
            nc.scalar.activation(
                ecol[:],
                u2r[:],
                func=mybir.ActivationFunctionType.Exp,
                scale=1.0 / 256.0,
            )
            ecols.append(ecol)

        # PE gather: one stationary matmul per 128-nnz block.
        psums = []
        for b in range(BPC):
            psum = psum_p.tile([128, NHI * NBLK], fp32, tag=f"ps{b}")
            for t in range(NBLK):
                oht = oh_tiles[(b, t // 32)]
                nc.tensor.matmul(
                    psum[:, NHI * t : NHI * (t + 1)],
                    oht[:, 128 * (t % 32) : 128 * (t % 32 + 1)],
                    ecols[b][:],
                    start=True,
                    stop=True,
                )
            psums.append(psum)

        # Quadrant select + windowed segment softmax + store.
        for b in range(BPC):
            sel = smallp.tile([128, NHI * NBLK], fp32, tag="sel")
            nc.vector.tensor_tensor(
                out=sel[:], in0=psums[b][:], in1=hm_tiles[b][:],
                op=mybir.AluOpType.mult,
            )
            c = smallp.tile([128, NBLK], fp32, tag="C")
            nc.vector.tensor_reduce(
                out=c[:],
                in_=sel[:].rearrange("p (t h) -> p t h", h=NHI),
                axis=mybir.AxisListType.X,
                op=mybir.AluOpType.add,
            )
            c3 = c[:].rearrange("p (q d) -> p q d", d=DEG)
            s = smallp.tile([128, 4], fp32, tag="S")
            nc.vector.tensor_reduce(
                out=s[:], in_=c3, axis=mybir.AxisListType.X, op=mybir.AluOpType.add
            )
            r = smallp.tile([128, 4], fp32, tag="R")
            nc.vector.reciprocal(r[:], s[:])
            o = smallp.tile([128, 128], fp32, tag="O")
            o3 = o[:].rearrange("p (q d) -> p q d", d=DEG)
            r3 = r[:].unsqueeze(2).broadcast_to((128, 4, DEG))
            nc.vector.tensor_tensor(out=o3, in0=c3, in1=r3, op=mybir.AluOpType.mult)
            nc.gpsimd.dma_start(out[b], o[:])

    nc.compile()
    return nc


def _prep_core_inputs(t2, idx_j, W2, v):
    import ml_dtypes

    bf16 = ml_dtypes.bfloat16
    fp8 = ml_dtypes.float8_e4m3fn
    g = (W2.T.astype(np.float64) @ v.astype(np.float64)).astype(np.float32)
    gcol = np.ascontiguousarray((g * 256.0).reshape(8, 128).T.astype(fp8))
    t2t = np.ascontiguousarray(t2.transpose(0, 2, 1).astype(fp8))

    # nnz (i, d) lands at C[p, t]: p = i % 128, t = 32*(i//128) + d
    i_arr = np.arange(N1)
    d_arr = np.arange(DEG)
    tt = (DEG * (i_arr[:, None] // 128) + d_arr[None, :])  # [512, 32]
    pp = np.broadcast_to((i_arr[:, None] % 128), (N1, DEG))

    j3 = np.asarray(idx_j).reshape(B, N1, DEG)
    in_maps = []
    eye = np.eye(CH, dtype=fp8)
    hvals = np.arange(NHI, dtype=np.int32)
    for c in range(NCORES):
        bb = slice(BPC * c, BPC * (c + 1))
        ohs = np.empty((BPC, CH, NBLK * 128), dtype=fp8)
        hms = np.empty((BPC, 128, NHI * NBLK), dtype=fp8)
        for lb in range(BPC):
            gb = BPC * c + lb
            jmat = np.empty((128, NBLK), dtype=np.int32)  # jmat[p, t] = J
            jmat[pp.ravel(), tt.ravel()] = j3[gb].ravel()
            lo = jmat % CH
            hi8 = jmat // CH  # 2t + h of the source layout
            hi = 4 * (hi8 % 2) + hi8 // 2  # h-major ecol column
            # ohs[lb][c_, 128t+p] = 1 iff c_ == lo[p, t]
            ohs[lb] = eye[:, lo.T].reshape(CH, NBLK * 128)
            hms[lb] = (hi[:, :, None] == hvals).astype(fp8).reshape(128, NHI * NBLK)
        in_maps.append(
            {
                "t2t": np.ascontiguousarray(t2t[bb]),
                "gcol": gcol,
                "oh": ohs,
                "hm": hms,
            }
        )
    return in_maps


def kernel(t1, t2, idx_b, idx_i, idx_j, W1, b1, W2, b2, v):
    from concourse.bass_utils import run_bass_kernel_spmd

    if "nc" not in _CACHE:
        _CACHE["nc"] = _build_program()
    nc = _CACHE["nc"]

    in_maps = _prep_core_inputs(
        np.asarray(t2, dtype=np.float32),
        np.asarray(idx_j),
        np.asarray(W2, dtype=np.float32),
        np.asarray(v, dtype=np.float32),
    )
    trace = bool(int(os.environ.get("KERNEL_TRACE", "0")))
    last_err = None
    for _attempt in range(3):
        try:
            res = run_bass_kernel_spmd(nc, in_maps, list(range(NCORES)), trace=trace)
            break
        except Exception as e:  # transient NRT_EXEC_UNIT_UNRECOVERABLE wedges
            last_err = e
    else:
        raise last_err
    _CACHE["last_results"] = res
    outs = []
    for r in res.results:
        o = r["out"].reshape(BPC, 128, 4, DEG)  # [b, p, q, d]
        o = o.transpose(0, 2, 1, 3).reshape(BPC * N1 * DEG)  # i = 128q + p
        outs.append(o)
    return np.concatenate(outs).astype(np.float32)


# revision 24
# speedup vs baseline: 1.0179x; 1.0179x over previous
"""Trainium2 Bass kernel for the sparse segment-softmax attention module.

Math: the reference computes, per nnz k,
    out[k] = segment_softmax((q1[b,i] + q2[b,j]) . v)  over segments (b, i).
Within a segment (fixed b, i), the q1[b,i].v term is constant and cancels in
softmax (shift invariance), as does the b2.v bias constant.  Hence
    out[k] = exp(u2[b, j_k]) / sum_{d in seg} exp(u2[b, j_d]),
    u2[b, n] = t2[b, n, :] . g,   g = W2^T v.

The baseline gathered exp(u2)[j] with the pool-engine IndirectCopy, which
is bound at ~28 ns per index per 16-partition group (~115 us for the 32768
gathers each NeuronCore owns).  ap_gather and dma_gather hit the same or
worse serial rates (measured).  This kernel instead does the gather on the
idle PE: the indices are kernel inputs, so the host uploads, per 128-nnz
block, a bf16 one-hot STATIONARY [128c x 128p] whose column p selects row
j%128; one LoadStationary+matmul against Ecol [128c x 4] (Ecol[c, hi] =
exp(u2[128*hi + c])) yields psum[p, 4*t+hi] = exp(u2[128*hi + L[p+128t]])
— 128 gathered candidate quads in ~130 PE cycles (~0.43 ns/value).  A DVE
multiply with a host-built hi-mask and a width-4 reduce selects the right
quadrant.  Per NeuronCore (2 batches): 256 stationary matmuls ~ 14 us on
the PE, overlapped with the one-hot upload stream.

Pipeline per core (2 of the 16 batches):
  - stream t2 shard (bf16); DVE mult by g-broadcast + ACT/DVE row-reduce
    -> u2acc [128, 4] (u2acc[p, t] = u2[128t + p]); ACT exp -> Ecol bf16.
  - 128 one-hot stationary matmuls per batch -> psum [128, 512].
  - DVE: psum * himask, reduce width-4 -> C [128, 128]; windowed segment
    softmax (sum 32, reciprocal, multiply); store.  Host applies the fixed
    inverse layout permutation.
"""

import os
from contextlib import ExitStack

import numpy as np

B = 16
N1 = 512
N2 = 512
F2 = 1024
DEG = 32
NNZ = B * N1 * DEG
NCORES = 8
BPC = B // NCORES  # batches per core
NBLK = 128  # one-hot blocks per batch (128 nnz each)
CH = 64  # one-hot contract height (j % CH selects the row)
NHI = 512 // CH  # quadrant count for the mask select

_CACHE: dict = {}


def _build_program():
    import concourse.bacc as bacc
    import concourse.mybir as mybir
    import concourse.tile as tile

    fp32 = mybir.dt.float32
    bf16 = mybir.dt.bfloat16
    fp8 = mybir.dt.float8e4

    nc = bacc.Bacc("TRN2", target_bir_lowering=False, debug=False)

    t2t = nc.dram_tensor("t2t", [BPC, F2, N2], fp8, kind="ExternalInput")
    gcol = nc.dram_tensor("gcol", [128, 8], fp8, kind="ExternalInput")
    # oh: one-hot stationaries, c-major: oh[b, c, 128t+p] = (J[p+128t] % CH == c)
    oh = nc.dram_tensor("oh", [BPC, CH, NBLK * 128], fp8, kind="ExternalInput")
    # hm: hi-quadrant mask, hm[p, NHI*t+hi] = (J[p+128t] // CH == hi)
    hm = nc.dram_tensor("hm", [BPC, 128, NHI * NBLK], fp8, kind="ExternalInput")
    out = nc.dram_tensor("out", [BPC, 128, 128], fp32, kind="ExternalOutput")

    with tile.TileContext(nc) as tc, ExitStack() as ctx:
        constp = ctx.enter_context(tc.tile_pool(name="const", bufs=1))
        t2p = ctx.enter_context(tc.tile_pool(name="t2p", bufs=4))
        prodp = ctx.enter_context(tc.tile_pool(name="prodp", bufs=3))
        ohp = ctx.enter_context(tc.tile_pool(name="ohp", bufs=1))
        smallp = ctx.enter_context(tc.tile_pool(name="small", bufs=2))
        psum_p = ctx.enter_context(tc.tile_pool(name="psg", bufs=1, space="PSUM"))
        psum_u = ctx.enter_context(tc.tile_pool(name="psu", bufs=2, space="PSUM"))

        # g columns (x256, fp8) at the head of the sync ring.
        g_sb = constp.tile([128, 8], fp8)
        nc.sync.dma_start(g_sb[:], gcol[:])
        hm_tiles = []
        for b in range(BPC):
            hm_t = constp.tile([128, NHI * NBLK], fp8, tag=f"hm{b}", name=f"hm{b}")
            nc.scalar.dma_start(hm_t[:], hm[b])
            hm_tiles.append(hm_t)

        # transposed-t2 stream on the scalar ring (1 MB fp8), batch 0
        # first; the sync ring is dedicated to the one-hot stream.
        t2_tiles = {}
        for b in range(BPC):
            for q in range(8):
                t2q = t2p.tile([128, N2], fp8, tag=f"t2_{b}_{q}", name=f"t2_{b}_{q}")
                nc.scalar.dma_start(t2q[:], t2t[b, 128 * q : 128 * (q + 1), :])
                t2_tiles[(b, q)] = t2q

        # One-hot stationaries: the sync ring carries ONLY this stream,
        # as four 1 MB chunk tiles per batch so each 32-block group of
        # matmuls starts as soon as its chunk lands.
        oh_tiles = {}
        OHCHUNK = NBLK * 128 // 4
        for b in range(BPC):
            eng = nc.sync if b == 0 else nc.scalar
            for h in range(4):
                oht = ohp.tile(
                    [CH, OHCHUNK], fp8, tag=f"oh{b}_{h}", name=f"oh{b}_{h}"
                )
                eng.dma_start(
                    oht[:], oh[b][:, h * OHCHUNK : (h + 1) * OHCHUNK]
                )
                oh_tiles[(b, h)] = oht

        # u2 on the PE: 256*u2[128t+p] = sum_q t2T[:, n].g -> upsum[p, t].
        ecols = []
        for b in range(BPC):
            upsum = psum_u.tile([128, 4], fp32, tag="upsum")
            for t in range(4):
                for q in range(8):
                    nc.tensor.matmul(
                        upsum[:, t : t + 1],
                        t2_tiles[(b, q)][:, 128 * t : 128 * (t + 1)],
                        g_sb[:, q : q + 1],
                        start=(q == 0),
                        stop=(q == 7),
                    )
            u2acc = smallp.tile([128, 4], fp32, tag="u2acc")
            nc.scalar.copy(u2acc[:], upsum[:])
            # re-lay u2acc [128, 4] -> [CH, NHI] h-major: u2[n] = u2acc[p, t]
            # with n = 128t + CH*h + c lands at column 4h + t
            u2r = smallp.tile([CH, NHI], fp32, tag=f"u2r{b}", name=f"u2r{b}")
            nc.gpsimd.dma_start(u2r[:, 0:4], u2acc[0:CH, :])
            nc.gpsimd.dma_start(u2r[:, 4:8], u2acc[CH:128, :])
            ecol = smallp.tile([CH, NHI], bf16, tag=f"ecol{b}", name=f"ecol{b}")
            nc.scalar.activation(
                ecol[:],
                u2r[:],
                func=mybir.ActivationFunctionType.Exp,
                scale=1.0 / 256.0,
            )
            ecols.append(ecol)

        # PE gather: one stationary matmul per 128-nnz block.
        psums = []
        for b in range(BPC):
            psum = psum_p.tile([128, NHI * NBLK], fp32, tag=f"ps{b}")
            for t in range(NBLK):
                oht = oh_tiles[(b, t // 32)]
                nc.tensor.matmul(
                    psum[:, NHI * t : NHI * (t + 1)],
                    oht[:, 128 * (t % 32) : 128 * (t % 32 + 1)],
                    ecols[b][:],
                    start=True,
                    stop=True,
                )
            psums.append(psum)

        # Quadrant select + windowed segment softmax + store.
        for b in range(BPC):
            sel = smallp.tile([128, NHI * NBLK], fp32, tag="sel")
            nc.vector.tensor_tensor(
                out=sel[:], in0=psums[b][:], in1=hm_tiles[b][:],
                op=mybir.AluOpType.mult,
            )
            c = smallp.tile([128, NBLK], fp32, tag="C")
            nc.vector.tensor_reduce(
                out=c[:],
                in_=sel[:].rearrange("p (t h) -> p t h", h=NHI),
                axis=mybir.AxisListType.X,
                op=mybir.AluOpType.add,
            )
            c3 = c[:].rearrange("p (q d) -> p q d", d=DEG)
            s = smallp.tile([128, 4], fp32, tag="S")
            nc.vector.tensor_reduce(
                out=s[:], in_=c3, axis=mybir.AxisListType.X, op=mybir.AluOpType.add
            )
            r = smallp.tile([128, 4], fp32, tag="R")
            nc.vector.reciprocal(r[:], s[:])
            o = smallp.tile([128, 128], fp32, tag="O")
            o3 = o[:].rearrange("p (q d) -> p q d", d=DEG)
            r3 = r[:].unsqueeze(2).broadcast_to((128, 4, DEG))
            nc.vector.tensor_tensor(out=o3, in0=c3, in1=r3, op=mybir.AluOpType.mult)
            nc.scalar.dma_start(out[b], o[:])

    nc.compile()
    return nc


def _prep_core_inputs(t2, idx_j, W2, v):
    import ml_dtypes

    bf16 = ml_dtypes.bfloat16
    fp8 = ml_dtypes.float8_e4m3fn
    g = (W2.T.astype(np.float64) @ v.astype(np.float64)).astype(np.float32)
    gcol = np.ascontiguousarray((g * 256.0).reshape(8, 128).T.astype(fp8))
    t2t = np.ascontiguousarray(t2.transpose(0, 2, 1).astype(fp8))

    # nnz (i, d) lands at C[p, t]: p = i % 128, t = 32*(i//128) + d
    i_arr = np.arange(N1)
    d_arr = np.arange(DEG)
    tt = (DEG * (i_arr[:, None] // 128) + d_arr[None, :])  # [512, 32]
    pp = np.broadcast_to((i_arr[:, None] % 128), (N1, DEG))

    j3 = np.asarray(idx_j).reshape(B, N1, DEG)
    in_maps = []
    eye = np.eye(CH, dtype=fp8)
    hvals = np.arange(NHI, dtype=np.int32)
    for c in range(NCORES):
        bb = slice(BPC * c, BPC * (c + 1))
        ohs = np.empty((BPC, CH, NBLK * 128), dtype=fp8)
        hms = np.empty((BPC, 128, NHI * NBLK), dtype=fp8)
        for lb in range(BPC):
            gb = BPC * c + lb
            jmat = np.empty((128, NBLK), dtype=np.int32)  # jmat[p, t] = J
            jmat[pp.ravel(), tt.ravel()] = j3[gb].ravel()
            lo = jmat % CH
            hi8 = jmat // CH  # 2t + h of the source layout
            hi = 4 * (hi8 % 2) + hi8 // 2  # h-major ecol column
            # ohs[lb][c_, 128t+p] = 1 iff c_ == lo[p, t]
            ohs[lb] = eye[:, lo.T].reshape(CH, NBLK * 128)
            hms[lb] = (hi[:, :, None] == hvals).astype(fp8).reshape(128, NHI * NBLK)
        in_maps.append(
            {
                "t2t": np.ascontiguousarray(t2t[bb]),
                "gcol": gcol,
                "oh": ohs,
                "hm": hms,
            }
        )
    return in_maps


def kernel(t1, t2, idx_b, idx_i, idx_j, W1, b1, W2, b2, v):
    from concourse.bass_utils import run_bass_kernel_spmd

    if "nc" not in _CACHE:
        _CACHE["nc"] = _build_program()
    nc = _CACHE["nc"]

    in_maps = _prep_core_inputs(
        np.asarray(t2, dtype=np.float32),
        np.asarray(idx_j),
        np.asarray(W2, dtype=np.float32),
        np.asarray(v, dtype=np.float32),
    )
    trace = bool(int(os.environ.get("KERNEL_TRACE", "0")))
    last_err = None
    for _attempt in range(3):
        try:
            res = run_bass_kernel_spmd(nc, in_maps, list(range(NCORES)), trace=trace)
            break
        except Exception as e:  # transient NRT_EXEC_UNIT_UNRECOVERABLE wedges
            last_err = e
    else:
        raise last_err
    _CACHE["last_results"] = res
    outs = []
    for r in res.results:
        o = r["out"].reshape(BPC, 128, 4, DEG)  # [b, p, q, d]
        o = o.transpose(0, 2, 1, 3).reshape(BPC * N1 * DEG)  # i = 128q + p
        outs.append(o)
    return np.concatenate(outs).astype(np.float32)


# revision 25
# speedup vs baseline: 1.0269x; 1.0088x over previous
"""Trainium2 Bass kernel for the sparse segment-softmax attention module.

Math: the reference computes, per nnz k,
    out[k] = segment_softmax((q1[b,i] + q2[b,j]) . v)  over segments (b, i).
Within a segment (fixed b, i), the q1[b,i].v term is constant and cancels in
softmax (shift invariance), as does the b2.v bias constant.  Hence
    out[k] = exp(u2[b, j_k]) / sum_{d in seg} exp(u2[b, j_d]),
    u2[b, n] = t2[b, n, :] . g,   g = W2^T v.

The baseline gathered exp(u2)[j] with the pool-engine IndirectCopy, bound
at ~28 ns per index per 16-partition group (~115 us for the 32768 gathers
each NeuronCore owns); ap_gather matches that rate and dma_gather's Q7
descriptor generation is ~8.4 ns/value of serial pool time (both measured
on HW).  This kernel instead gathers on the otherwise-idle PE: indices are
kernel inputs, so the host uploads, per 128-nnz block, an fp8 one-hot
STATIONARY [64c x 128p] whose column p selects row j%64.  One
LoadStationary+matmul against Ecol [64c x 8] (Ecol[c, 4h+t] =
exp(u2[128t + 64h + c]), bf16) yields psum[p, 8t':8t'+8] — each output
partition picks its own table row, 128 value-octets in ~70 PE cycles
(~0.5 ns/value).  A DVE multiply with a host-built fp8 hi-mask and a
width-8 reduce selects the right octet lane.

Pipeline per core (2 of the 16 batches):
  - u2 on the PE: host-transposed fp8 t2 chunks as stationaries against
    g-columns (x256, fp8; the exp fuses a 1/256 scale) -> upsum [128, 4];
    ACT copy, two small SWDGE DMAs re-lay to [64, 8], ACT exp -> Ecol.
  - one-hot stream (1 MB fp8 per batch) split across the sync and scalar
    HWDGE rings in 256 KB chunks so each 32-block matmul group starts as
    soon as its chunk lands (the stream is the kernel's critical path).
  - 128 gather matmuls per batch -> psum [128, 1024].
  - DVE: psum * himask, width-8 reduce -> C [128, 128]; windowed segment
    softmax (sum 32, reciprocal, multiply); store.  The host applies the
    fixed inverse layout permutation (nnz (i, d) sits at C[i%128,
    32*(i//128)+d]).
"""

import os
from contextlib import ExitStack

import numpy as np

B = 16
N1 = 512
N2 = 512
F2 = 1024
DEG = 32
NNZ = B * N1 * DEG
NCORES = 8
BPC = B // NCORES  # batches per core
NBLK = 128  # one-hot blocks per batch (128 nnz each)
CH = 64  # one-hot contract height (j % CH selects the row)
NHI = 512 // CH  # quadrant count for the mask select

_CACHE: dict = {}


def _build_program():
    import concourse.bacc as bacc
    import concourse.mybir as mybir
    import concourse.tile as tile

    fp32 = mybir.dt.float32
    bf16 = mybir.dt.bfloat16
    fp8 = mybir.dt.float8e4

    nc = bacc.Bacc("TRN2", target_bir_lowering=False, debug=False)

    t2t = nc.dram_tensor("t2t", [BPC, F2, N2], fp8, kind="ExternalInput")
    gcol = nc.dram_tensor("gcol", [128, 8], fp8, kind="ExternalInput")
    # oh: one-hot stationaries, c-major: oh[b, c, 128t+p] = (J[p+128t] % CH == c)
    oh = nc.dram_tensor("oh", [BPC, CH, NBLK * 128], fp8, kind="ExternalInput")
    # hm: hi-quadrant mask, hm[p, NHI*t+hi] = (J[p+128t] // CH == hi)
    hm = nc.dram_tensor("hm", [BPC, 128, NHI * NBLK], fp8, kind="ExternalInput")
    out = nc.dram_tensor("out", [BPC, 128, 128], fp32, kind="ExternalOutput")

    with tile.TileContext(nc) as tc, ExitStack() as ctx:
        constp = ctx.enter_context(tc.tile_pool(name="const", bufs=1))
        t2p = ctx.enter_context(tc.tile_pool(name="t2p", bufs=4))
        ohp = ctx.enter_context(tc.tile_pool(name="ohp", bufs=1))
        smallp = ctx.enter_context(tc.tile_pool(name="small", bufs=2))
        psum_p = ctx.enter_context(tc.tile_pool(name="psg", bufs=1, space="PSUM"))
        psum_u = ctx.enter_context(tc.tile_pool(name="psu", bufs=2, space="PSUM"))

        # g columns (x256, fp8) at the head of the sync ring.
        g_sb = constp.tile([128, 8], fp8)
        nc.sync.dma_start(g_sb[:], gcol[:])
        hm_tiles = []
        for b in range(BPC):
            hm_t = constp.tile([128, NHI * NBLK], fp8, tag=f"hm{b}", name=f"hm{b}")
            nc.scalar.dma_start(hm_t[:], hm[b])
            hm_tiles.append(hm_t)

        # transposed-t2 stream on the scalar ring (1 MB fp8), batch 0
        # first; the sync ring is dedicated to the one-hot stream.
        t2_tiles = {}
        for b in range(BPC):
            for q in range(8):
                t2q = t2p.tile([128, N2], fp8, tag=f"t2_{b}_{q}", name=f"t2_{b}_{q}")
                nc.scalar.dma_start(t2q[:], t2t[b, 128 * q : 128 * (q + 1), :])
                t2_tiles[(b, q)] = t2q

        # One-hot stationaries: the sync ring carries ONLY this stream,
        # as four 1 MB chunk tiles per batch so each 32-block group of
        # matmuls starts as soon as its chunk lands.
        oh_tiles = {}
        OHCHUNK = NBLK * 128 // 4
        for b in range(BPC):
            eng = nc.sync if b == 0 else nc.scalar
            for h in range(4):
                oht = ohp.tile(
                    [CH, OHCHUNK], fp8, tag=f"oh{b}_{h}", name=f"oh{b}_{h}"
                )
                eng.dma_start(
                    oht[:], oh[b][:, h * OHCHUNK : (h + 1) * OHCHUNK]
                )
                oh_tiles[(b, h)] = oht

        # u2 on the PE: 256*u2[128t+p] = sum_q t2T[:, n].g -> upsum[p, t].
        ecols = []
        for b in range(BPC):
            upsum = psum_u.tile([128, 4], fp32, tag="upsum")
            for t in range(4):
                for q in range(8):
                    nc.tensor.matmul(
                        upsum[:, t : t + 1],
                        t2_tiles[(b, q)][:, 128 * t : 128 * (t + 1)],
                        g_sb[:, q : q + 1],
                        start=(q == 0),
                        stop=(q == 7),
                    )
            u2acc = smallp.tile([128, 4], fp32, tag="u2acc")
            nc.scalar.copy(u2acc[:], upsum[:])
            # re-lay u2acc [128, 4] -> [CH, NHI] h-major: u2[n] = u2acc[p, t]
            # with n = 128t + CH*h + c lands at column 4h + t
            u2r = smallp.tile([CH, NHI], fp32, tag=f"u2r{b}", name=f"u2r{b}")
            nc.gpsimd.dma_start(u2r[:, 0:4], u2acc[0:CH, :])
            nc.gpsimd.dma_start(u2r[:, 4:8], u2acc[CH:128, :])
            ecol = smallp.tile([CH, NHI], bf16, tag=f"ecol{b}", name=f"ecol{b}")
            nc.scalar.activation(
                ecol[:],
                u2r[:],
                func=mybir.ActivationFunctionType.Exp,
                scale=1.0 / 256.0,
            )
            ecols.append(ecol)

        # PE gather: one stationary matmul per 128-nnz block.
        psums = []
        for b in range(BPC):
            psum = psum_p.tile([128, NHI * NBLK], fp32, tag=f"ps{b}")
            for t in range(NBLK):
                oht = oh_tiles[(b, t // 32)]
                nc.tensor.matmul(
                    psum[:, NHI * t : NHI * (t + 1)],
                    oht[:, 128 * (t % 32) : 128 * (t % 32 + 1)],
                    ecols[b][:],
                    start=True,
                    stop=True,
                )
            psums.append(psum)

        # Quadrant select + windowed segment softmax + store.
        for b in range(BPC):
            sel = smallp.tile([128, NHI * NBLK], fp32, tag="sel")
            nc.vector.tensor_tensor(
                out=sel[:], in0=psums[b][:], in1=hm_tiles[b][:],
                op=mybir.AluOpType.mult,
            )
            c = smallp.tile([128, NBLK], fp32, tag="C")
            nc.vector.tensor_reduce(
                out=c[:],
                in_=sel[:].rearrange("p (t h) -> p t h", h=NHI),
                axis=mybir.AxisListType.X,
                op=mybir.AluOpType.add,
            )
            c3 = c[:].rearrange("p (q d) -> p q d", d=DEG)
            s = smallp.tile([128, 4], fp32, tag="S")
            nc.vector.tensor_reduce(
                out=s[:], in_=c3, axis=mybir.AxisListType.X, op=mybir.AluOpType.add
            )
            r = smallp.tile([128, 4], fp32, tag="R")
            nc.vector.reciprocal(r[:], s[:])
            o = smallp.tile([128, 128], fp32, tag="O")
            o3 = o[:].rearrange("p (q d) -> p q d", d=DEG)
            r3 = r[:].unsqueeze(2).broadcast_to((128, 4, DEG))
            nc.vector.tensor_tensor(out=o3, in0=c3, in1=r3, op=mybir.AluOpType.mult)
            nc.scalar.dma_start(out[b], o[:])

    nc.compile()
    return nc


def _prep_core_inputs(t2, idx_j, W2, v):
    import ml_dtypes

    bf16 = ml_dtypes.bfloat16
    fp8 = ml_dtypes.float8_e4m3fn
    g = (W2.T.astype(np.float64) @ v.astype(np.float64)).astype(np.float32)
    gcol = np.ascontiguousarray((g * 256.0).reshape(8, 128).T.astype(fp8))
    t2t = np.ascontiguousarray(t2.transpose(0, 2, 1).astype(fp8))

    # nnz (i, d) lands at C[p, t]: p = i % 128, t = 32*(i//128) + d
    i_arr = np.arange(N1)
    d_arr = np.arange(DEG)
    tt = (DEG * (i_arr[:, None] // 128) + d_arr[None, :])  # [512, 32]
    pp = np.broadcast_to((i_arr[:, None] % 128), (N1, DEG))

    j3 = np.asarray(idx_j).reshape(B, N1, DEG)
    in_maps = []
    eye = np.eye(CH, dtype=fp8)
    hvals = np.arange(NHI, dtype=np.int32)
    for c in range(NCORES):
        bb = slice(BPC * c, BPC * (c + 1))
        ohs = np.empty((BPC, CH, NBLK * 128), dtype=fp8)
        hms = np.empty((BPC, 128, NHI * NBLK), dtype=fp8)
        for lb in range(BPC):
            gb = BPC * c + lb
            jmat = np.empty((128, NBLK), dtype=np.int32)  # jmat[p, t] = J
            jmat[pp.ravel(), tt.ravel()] = j3[gb].ravel()
            lo = jmat % CH
            hi8 = jmat // CH  # 2t + h of the source layout
            hi = 4 * (hi8 % 2) + hi8 // 2  # h-major ecol column
            # ohs[lb][c_, 128t+p] = 1 iff c_ == lo[p, t]
            ohs[lb] = eye[:, lo.T].reshape(CH, NBLK * 128)
            hms[lb] = (hi[:, :, None] == hvals).astype(fp8).reshape(128, NHI * NBLK)
        in_maps.append(
            {
                "t2t": np.ascontiguousarray(t2t[bb]),
                "gcol": gcol,
                "oh": ohs,
                "hm": hms,
            }
        )
    return in_maps


def kernel(t1, t2, idx_b, idx_i, idx_j, W1, b1, W2, b2, v):
    from concourse.bass_utils import run_bass_kernel_spmd

    if "nc" not in _CACHE:
        _CACHE["nc"] = _build_program()
    nc = _CACHE["nc"]

    in_maps = _prep_core_inputs(
        np.asarray(t2, dtype=np.float32),
        np.asarray(idx_j),
        np.asarray(W2, dtype=np.float32),
        np.asarray(v, dtype=np.float32),
    )
    trace = bool(int(os.environ.get("KERNEL_TRACE", "0")))
    last_err = None
    for _attempt in range(3):
        try:
            res = run_bass_kernel_spmd(nc, in_maps, list(range(NCORES)), trace=trace)
            break
        except Exception as e:  # transient NRT_EXEC_UNIT_UNRECOVERABLE wedges
            last_err = e
    else:
        raise last_err
    _CACHE["last_results"] = res
    outs = []
    for r in res.results:
        o = r["out"].reshape(BPC, 128, 4, DEG)  # [b, p, q, d]
        o = o.transpose(0, 2, 1, 3).reshape(BPC * N1 * DEG)  # i = 128q + p
        outs.append(o)
    return np.concatenate(outs).astype(np.float32)
